# revision 9
# baseline (speedup 1.0000x reference)
"""DenseVariational bass kernel for TRN2 (8 NeuronCores).

Problem: out[s,b,o] = sum_i input[s,b,i] * (mu[o,i] + softplus(rho[o,i])*eps_w[s,o,i])
                      + bias_mu[o] + softplus(bias_rho[o])*eps_b[s,o]
  S=32 samples, B=256, IN=OUT=1024, fp32 in/out.

Sharding: samples split 4-per-core across 8 cores; mu/rho replicated.

Per-core device program (transposed-output form; PSUM = [o, b]):
  - All large loads are Pool-engine (SWDGE) DMAs that cast fp32->bf16 in
    flight, halving HBM-side transfer cost charged to the DMA engines.
  - sigma.T = softplus(rho.T) = Ln(Exp(rho.T)+1) on ScalarE over bf16,
    resident in SBUF; mu.T resident in SBUF (bf16).
  - per sample: stream eps_w[s].T chunks (bf16); DVE computes
    W.T = sigma.T*eps.T + mu.T in place; PE accumulates
    psum[ob] += W.T[kt,ob].T @ X.T[kt] with bf16 operands (1 cycle/row).
  - PSUM -> SBUF bf16 on ScalarE with a per-partition fp32 bias operand
    (bias add for free; o is the partition dim in this form).
  - output written transposed+blocked in bf16; host upcasts + unpermutes.

Host pre-arranges layouts (pure data movement, part of sharding):
  xt[s][p, kt*256+b]  = input[s, b, kt*128+p]
  epst[s][i, o]       = eps_w[s, o, i]
  mut/rhot[i, o]      = mu/rho[o, i]
  epsb_po[p, s*8+ob]  = eps_b[s, ob*128+p]
  bmu_po/brho_po[p, ob] = bias_mu/bias_rho[ob*128+p]
  output yt[s][p, ob*256+b] = out[s, b, ob*128+p]
"""

import numpy as np

import concourse.bass as bass
import concourse.mybir as mybir
import concourse.tile as tile
from concourse import bacc
from concourse.bass_utils import run_bass_kernel_spmd

# Problem constants (hardcoded per harness contract)
S, B, IN, OUT = 32, 256, 1024, 1024
NCORES = 8
SL = S // NCORES          # samples per core = 4
P = 128
KT = IN // P              # 8 k-tiles
OB = OUT // P             # 8 output-row blocks
FP32 = mybir.dt.float32
BF16 = mybir.dt.bfloat16
ActF = mybir.ActivationFunctionType
ADD = mybir.AluOpType.add

# per-sample eps chunking in k-tiles: sample 0 leads with small chunks so
# compute ramps with the setup stream; the last sample ends with a 1-k-tile
# chunk to shrink the serial tail.
CHUNKS = [
    [(0, 1), (1, 2), (2, 4), (4, 6), (6, 8)],
    [(0, 2), (2, 4), (4, 6), (6, 8)],
    [(0, 2), (2, 4), (4, 6), (6, 8)],
    [(0, 3), (3, 5), (5, 7), (7, 8)],
]

_cached = None


def build_bass(repeat: int = 1):
    nc = bacc.Bacc(
        "TRN2",
        target_bir_lowering=False,
        debug=False,
        enable_asserts=False,
        num_devices=NCORES,
    )

    xt = nc.dram_tensor("xt", (SL, P, KT * B), FP32, kind="ExternalInput").ap()
    epst = nc.dram_tensor("epst", (SL, IN, OUT), FP32, kind="ExternalInput").ap()
    mut = nc.dram_tensor("mut", (IN, OUT), FP32, kind="ExternalInput").ap()
    rhot = nc.dram_tensor("rhot", (IN, OUT), FP32, kind="ExternalInput").ap()
    bmu_po = nc.dram_tensor("bmu_po", (P, OB), FP32, kind="ExternalInput").ap()
    brho_po = nc.dram_tensor("brho_po", (P, OB), FP32, kind="ExternalInput").ap()
    epsb_po = nc.dram_tensor("epsb_po", (P, SL * OB), FP32, kind="ExternalInput").ap()
    yt = nc.dram_tensor("yt", (SL, P, OB * B), BF16, kind="ExternalOutput").ap()

    mut_r = mut.rearrange("(kt p) o -> p kt o", p=P)
    rhot_r = rhot.rearrange("(kt p) o -> p kt o", p=P)

    with tile.TileContext(nc) as tc:
        with (
            tc.tile_pool(name="persist", bufs=1) as persist,
            tc.tile_pool(name="eps", bufs=10) as eps_pool,
            tc.tile_pool(name="xtp", bufs=3) as xt_pool,
            tc.tile_pool(name="outp", bufs=2) as out_pool,
            tc.tile_pool(name="psum", bufs=2, space="PSUM") as psum_pool,
        ):
            mu_sb = persist.tile([P, KT, OUT], BF16)
            sig_sb = persist.tile([P, KT, OUT], BF16)
            sigb_po = persist.tile([P, OB], FP32)
            bmu_sb = persist.tile([P, OB], FP32)
            bias_sb = persist.tile([P, SL * OB], FP32)

            # small fp32 bias DMAs ride the (otherwise idle) SP HWDGE path
            nc.sync.dma_start(out=sigb_po[:], in_=brho_po[:])
            nc.sync.dma_start(out=bmu_sb[:], in_=bmu_po[:])
            nc.sync.dma_start(out=bias_sb[:], in_=epsb_po[:])
            nc.scalar.activation(sigb_po[:], sigb_po[:], ActF.Exp)
            nc.scalar.activation(sigb_po[:], sigb_po[:], ActF.Ln, bias=1.0)

            # setup loads interleaved with sample-0's eps stream so compute
            # ramps immediately; every large load is a Pool SWDGE cast-DMA
            # (fp32 -> bf16). Emission order on the Pool queue is the DMA
            # device order, so: rho_k / eps0-chunk / mu / x0 round-robin,
            # with softplus(rho_k) on ScalarE chained per k-tile.
            xt_tiles = {}
            xt_tiles[(0, 0)] = xt_pool.tile([P, KT * B], BF16, tag="xt",
                                            name="xt_sb0")
            eps0_tiles = {}

            def load_eps0(c):
                k0, k1 = CHUNKS[0][c]
                t = eps_pool.tile([P, k1 - k0, OUT], BF16, tag="eps",
                                  name=f"eps_0_0_{c}")
                nc.gpsimd.dma_start(
                    out=t[:],
                    in_=epst[0, k0 * P:k1 * P, :].rearrange(
                        "(kt p) o -> p kt o", p=P
                    ),
                )
                eps0_tiles[c] = t

            def softplus(k0, k1):
                # softplus(x) = Ln(Exp(x)+1), in place over bf16
                nc.scalar.activation(
                    sig_sb[:, k0:k1, :], sig_sb[:, k0:k1, :], ActF.Exp
                )
                nc.scalar.activation(
                    sig_sb[:, k0:k1, :], sig_sb[:, k0:k1, :], ActF.Ln, bias=1.0
                )

            # k-tile-granular rho loads + softplus; eps0/mu/x0 slotted between
            nc.gpsimd.dma_start(out=sig_sb[:, 0:1, :], in_=rhot_r[:, 0:1, :])
            load_eps0(0)
            softplus(0, 1)
            nc.gpsimd.dma_start(out=mu_sb[:, 0:1, :], in_=mut_r[:, 0:1, :])
            nc.gpsimd.dma_start(out=sig_sb[:, 1:2, :], in_=rhot_r[:, 1:2, :])
            load_eps0(1)
            softplus(1, 2)
            nc.gpsimd.dma_start(out=mu_sb[:, 1:2, :], in_=mut_r[:, 1:2, :])
            nc.gpsimd.dma_start(out=xt_tiles[(0, 0)][:], in_=xt[0])
            nc.gpsimd.dma_start(out=sig_sb[:, 2:4, :], in_=rhot_r[:, 2:4, :])
            load_eps0(2)
            softplus(2, 4)
            nc.gpsimd.dma_start(out=mu_sb[:, 2:4, :], in_=mut_r[:, 2:4, :])
            nc.gpsimd.dma_start(out=sig_sb[:, 4:6, :], in_=rhot_r[:, 4:6, :])
            load_eps0(3)
            softplus(4, 6)
            nc.gpsimd.dma_start(out=mu_sb[:, 4:6, :], in_=mut_r[:, 4:6, :])
            nc.gpsimd.dma_start(out=sig_sb[:, 6:8, :], in_=rhot_r[:, 6:8, :])
            load_eps0(4)
            softplus(6, 8)
            nc.gpsimd.dma_start(out=mu_sb[:, 6:8, :], in_=mut_r[:, 6:8, :])

            # bias_sb[p, s*OB+ob] = bmu + softplus(brho) * eps_b
            for s in range(SL):
                sl_ = bias_sb[:, s * OB:(s + 1) * OB]
                nc.vector.tensor_mul(out=sl_, in0=sl_, in1=sigb_po[:])
                nc.vector.tensor_add(out=sl_, in0=sl_, in1=bmu_sb[:])

            # ---- main loop over local samples ----
            for rep in range(repeat):
              for s in range(SL):
                if s > 0 or rep > 0:
                    t = xt_pool.tile([P, KT * B], BF16, tag="xt",
                                     name=f"xt_sb{rep}_{s}")
                    nc.gpsimd.dma_start(out=t[:], in_=xt[s])
                    xt_tiles[(rep, s)] = t
                xt_sb = xt_tiles[(rep, s)]

                psums = [
                    psum_pool.tile([P, 2 * B], FP32, tag=f"pb{t}", name=f"psum_{t}")
                    for t in range(OB // 2)
                ]

                for c, (k0, k1) in enumerate(CHUNKS[s]):
                    kg = k1 - k0
                    if s == 0 and rep == 0:
                        eps_sb = eps0_tiles[c]
                    else:
                        eps_sb = eps_pool.tile(
                            [P, kg, OUT], BF16, tag="eps",
                            name=f"eps_{rep}_{s}_{c}",
                        )
                        nc.gpsimd.dma_start(
                            out=eps_sb[:],
                            in_=epst[s, k0 * P:k1 * P, :].rearrange(
                                "(kt p) o -> p kt o", p=P
                            ),
                        )
                    # W.T chunk = sigma.T * eps.T + mu.T, in place (bf16)
                    nc.vector.tensor_mul(
                        out=eps_sb[:], in0=eps_sb[:], in1=sig_sb[:, k0:k1, :],
                    )
                    nc.vector.tensor_add(
                        out=eps_sb[:], in0=eps_sb[:], in1=mu_sb[:, k0:k1, :],
                    )
                    for kt in range(k0, k1):
                        kk = kt - k0
                        rhs = xt_sb[:, kt * B:(kt + 1) * B]
                        for ob in range(OB):
                            t, j = divmod(ob, 2)
                            # start=True clears the has_written flags of the
                            # WHOLE psum bank, so only the bank's first matmul
                            # may carry it; the j=1 group's first write still
                            # overwrites (not accumulates) because its
                            # per-element flags were cleared by the j=0 start.
                            nc.tensor.matmul(
                                psums[t][:, j * B:(j + 1) * B],
                                eps_sb[:, kk, ob * P:(ob + 1) * P],
                                rhs,
                                start=(kt == 0 and j == 0),
                                stop=(kt == KT - 1),
                                skip_group_check=True,
                            )

                out_sb = out_pool.tile([P, OB * B], BF16)
                for ob in range(OB):
                    t, j = divmod(ob, 2)
                    nc.scalar.activation(
                        out_sb[:, ob * B:(ob + 1) * B],
                        psums[t][:, j * B:(j + 1) * B],
                        ActF.Identity,
                        bias=bias_sb[:, s * OB + ob: s * OB + ob + 1],
                    )
                    if s == SL - 1 and rep == repeat - 1 and j == 1:
                        # tail: stream the last sample's output per psum-bank
                        nc.sync.dma_start(
                            out=yt[s, :, (ob - 1) * B:(ob + 1) * B],
                            in_=out_sb[:, (ob - 1) * B:(ob + 1) * B],
                        )
                if not (s == SL - 1 and rep == repeat - 1):
                    nc.sync.dma_start(out=yt[s], in_=out_sb[:])

    nc.compile()
    return nc


def _prepare_in_maps(input, weight_mu, weight_rho, bias_mu, bias_rho, eps_w, eps_b):
    f = np.float32
    input = np.ascontiguousarray(input, dtype=f)
    eps_w = np.ascontiguousarray(eps_w, dtype=f)
    eps_b = np.asarray(eps_b, f)

    # xt[s, p, kt*B + b] = input[s, b, kt*P + p]
    xt_all = np.ascontiguousarray(
        input.reshape(S, B, KT, P).transpose(0, 3, 2, 1).reshape(S, P, KT * B)
    )
    # epst[s, i, o] = eps_w[s, o, i]
    epst_all = np.ascontiguousarray(eps_w.transpose(0, 2, 1))
    mut = np.ascontiguousarray(np.asarray(weight_mu, f).T)
    rhot = np.ascontiguousarray(np.asarray(weight_rho, f).T)
    bmu_po = np.ascontiguousarray(np.asarray(bias_mu, f).reshape(OB, P).T)
    brho_po = np.ascontiguousarray(np.asarray(bias_rho, f).reshape(OB, P).T)

    in_maps = []
    for c in range(NCORES):
        sl = slice(c * SL, (c + 1) * SL)
        epsb_po = np.ascontiguousarray(
            eps_b[sl].reshape(SL, OB, P).transpose(2, 0, 1).reshape(P, SL * OB)
        )
        in_maps.append({
            "xt": np.ascontiguousarray(xt_all[sl]),
            "epst": np.ascontiguousarray(epst_all[sl]),
            "mut": mut,
            "rhot": rhot,
            "bmu_po": bmu_po,
            "brho_po": brho_po,
            "epsb_po": epsb_po,
        })
    return in_maps


def run(trace=False, trace_cores=None, **inputs):
    global _cached
    if _cached is None:
        _cached = build_bass()
    nc = _cached
    in_maps = _prepare_in_maps(**inputs)
    res = run_bass_kernel_spmd(
        nc,
        in_maps,
        core_ids=list(range(NCORES)),
        trace=trace,
        trace_cores=trace_cores,
    )
    # yt[s, p, ob*B+b] = out[s, b, ob*P+p] -> upcast, unpermute and gather
    outs = []
    for r in res.results:
        y = np.asarray(r["yt"]).astype(np.float32)
        y = y.reshape(SL, P, OB, B).transpose(0, 3, 2, 1).reshape(SL, B, OUT)
        outs.append(y)
    return np.ascontiguousarray(np.concatenate(outs, axis=0)), res


def kernel(**inputs) -> np.ndarray:
    out, _ = run(trace=False, **inputs)
    return out
